# revision 79
# baseline (speedup 1.0000x reference)
"""Trainium2 Bass kernel for nn_Attention_46995532153449.

Module: qkv = x @ w_qkv; per-head scores = q k^T * hd^-0.5; softmax over the
HEAD axis (axis=1); attn = probs @ v; out = attn @ w_proj + b_proj.

Shapes: B=2, T=2048, D=1024, H=16, HD=64.

Sharding: data-parallel over (batch, query-block): core c handles batch c//4
and queries [(c%4)*512, (c%4+1)*512). The head-axis softmax is local (each
core holds all 16 heads for its query slice). K/V for the whole batch are
recomputed per core (collectives are priced far above their compute saving
by the cost model, so no cross-core exchange).

Structure (all chosen against the TimelineSim cost model):
  - host feeds x^T fp16 with columns ROTATED so the core's own 512 queries
    are columns 0:512 (one SPMD program, per-core data). Key order is a
    rotation, which attention is invariant to.
  - attention runs as two passes over the 16 key chunks (qh = 256-query
    halves) to fit PSUM. Pass A also produces K/V, software-pipelined as
    per-chunk lookahead filler emitted BETWEEN a chunk's score groups so
    the PE never stalls on the single-buffered scores tile's exp
    round-trips; PV lags one chunk.
  - pass A additionally PRECOMPUTES qh1's P (exp'd, normalized probs) for
    chunks 10/11 into a persistent tile: the score head-pairs ride the
    filler rotation through kvps PSUM slots (kc 5..9), the head-sum at
    kc=8/11 and the normalize-multiply halves at kc 9/10/12/13 land in
    DVE/Pool slack. Pass B then runs only 14 full chunks and its drain has
    dependency-free PV work.
  - pass A's last two chunks (no fillers left) run with double-buffered
    scores using kvps' freed banks, so they pace at ACT speed.
  - pass A PV uses the attn[q, d] orientation (half the PE cycles);
    pass B PV accumulates DIRECTLY in attn^T[d, q] ([64-partition, 256]
    matmuls into per-head partition halves): 2x the PE cycles, but pass B
    is ACT-paced with PE slack, and this deletes the evac+xbar-transpose
    serial chain before the output projection. PSUM accumulation across
    chunks uses pre-zeroed banks + start=False (column-split groups must
    not use start=True, which clears the whole partition row of a bank).
  - softmax head-sum: first tree level split [0:2] Pool / [2:4] DVE (both
    need only exp groups g0+g2) / [4:8] DVE (g1+g3) so the post-last-exp
    serial chain is short; then l2..l4 + reciprocal on DVE; P = E*R split
    DVE(13 pass A / 12 pass B)/GpSimd(rest). Pass B PV lags THREE chunks
    so the chain never stalls the in-order PE. GPSIMD cannot touch PSUM
    (verifier rule) — all PSUM-side copies/memsets are DVE/ACT.
  - pass B drain: PV for precomputed chunks 10/11 (stop flag on the last)
    covers chunks 14/15's softmax chains; the projection PSUM opens in
    scB's freed banks so proj runs straight off the drain with the PE
    p-state still hot; qh0's proj groups go first (their aT half is ready
    since pass A) covering the qh1 attn^T evacuation.
  - startup: DMA ring begins with quarter-size wq/xT pieces and Q's first
    e-chunk runs in column halves, so the PE starts ~3.5us in and streams.
    Per-DMA SP sequencer time is 565ns: more/smaller pieces starve the
    mid-Q stream (measured) — this split is the tuned balance.
  - pass A PV runs qc-outer so acc half 0 finishes early; each att half's
    evac + xbar transpose is emitted as soon as its half lands.

Measured: TimelineSim 272.9us/core (the harness metric), from 281.7us at
session start (382.1us original); hardware-run max rel err 6.7e-4 vs a
float64 reference.
Rejected avenues (measured): AllGather of K/V (cost model: 15us overhead +
40GB/s effective -> ~225us for 8MB, dwarfing the 82us of saved matmul);
remote_dma K/V exchange (unmodeled in no_exec TimelineSim -- the metric --
and RemoteDMA unsupported without MultiCoreSim); fp8-e4m3 DoubleRow scores
(empirically 2.8e-2 max rel err vs the 2e-2 gate); K/V-evac on DVE
(starves early-chunk DVE work); per-d-chunk attn^T evac copies (tile-level
write tracking serializes them -- batch into 2 wide copies instead).
"""

import numpy as np

import concourse.bacc as bacc
import concourse.mybir as mybir
import concourse.tile as tile
from concourse import bass_utils

B, T, D, H = 2, 2048, 1024, 16
HD = D // H          # 64
SCALE = HD ** -0.5   # 0.125
NCORES = 8
QS = B * T // NCORES  # 512 queries per core
QH = QS // 2          # 256-query halves (PSUM budget)
DC = D // 128         # 8 d/e chunks of 128
TC = T // 128         # 16 key chunks of 128

F16 = mybir.dt.float16
F32 = mybir.dt.float32
ADD = mybir.AluOpType.add
MULT = mybir.AluOpType.mult
EXP = mybir.ActivationFunctionType.Exp

_CACHED_NC = None


def _build_nc():
    nc = bacc.Bacc(
        "TRN2", target_bir_lowering=False, debug=False, enable_asserts=False
    )

    xt_d = nc.dram_tensor("xt", [D, T], F16, kind="ExternalInput").ap()
    wq_d = nc.dram_tensor("wq", [D, D], F16, kind="ExternalInput").ap()
    wk_d = nc.dram_tensor("wk", [D, D], F16, kind="ExternalInput").ap()
    wv_d = nc.dram_tensor("wv", [D, D], F16, kind="ExternalInput").ap()
    wp_d = nc.dram_tensor("wp", [D, D], F16, kind="ExternalInput").ap()
    bias_d = nc.dram_tensor("bias", [128, D], F32, kind="ExternalInput").ap()
    out_d = nc.dram_tensor("out", [QS, D], F32, kind="ExternalOutput").ap()

    def chunked(ap):  # [(c p), f] -> [p, c, f]
        return ap.rearrange("(c p) f -> p c f", p=128)

    xt_ch = chunked(xt_d)
    wq_ch = chunked(wq_d)
    out_ch = chunked(out_d)

    with tile.TileContext(nc) as tc:
        with tc.tile_pool(name="persist", bufs=1) as pp:
            kT = pp.tile([128, DC, T], F16)      # k^T: [e, t], e-chunk major
            v_sb = pp.tile([128, TC, D], F16)    # v: [t, e], t-chunk major
            # zero-padded q^T: for head pair pr and query half qh, columns
            # [0:QH] hold head 2pr's q^T at partitions 0:64 (zeros below),
            # columns [QH:2QH] hold head 2pr+1's at partitions 64:128, so
            # every scores matmul is a full-128-partition K=128 matmul.
            qpad = pp.tile([128, DC, 2, 2 * QH], F16)
            att = pp.tile([128, 2, D], F16)      # attn [q, d], per-qh reuse
            aT = pp.tile([128, DC, QS], F16)     # attn^T [d, q]
            # P (normalized probs) for qh1 chunks 10/11, precomputed in pass
            # A's ACT/DVE slack so pass B runs only 14 full chunks and its
            # drain has dependency-free PV work (pass B processes these
            # chunks' PV last — accumulation order is commutative)
            ppre = pp.tile([128, 2, H, QH], F16)
            rpre = pp.tile([128, 2, 1, QH], F16)  # 1/S for ppre chunks

            nc.gpsimd.memset(qpad, 0.0)

            with tc.tile_pool(name="pA", bufs=1) as pA:
                xT = pA.tile([128, DC, T], F16)
                wk_sb = pA.tile([128, DC, D], F16)
                wv_sb = pA.tile([128, DC, D], F16)

                with (
                    tc.tile_pool(name="qpool", bufs=1) as qp,
                    tc.tile_pool(name="qpsum", bufs=4, space="PSUM") as qpsum,
                ):
                    wq_sb = qp.tile([128, DC, D], F16)
                    # DMA ring order: first wq e-chunk -> own x^T in two
                    # pieces -> rest of wq -> wk -> wv -> remaining x^T
                    # pieces. Q's ej=0 starts after just wq0+xT-own-half;
                    # later ej's consume wq chunks as they stream in.
                    nc.sync.dma_start(
                        wq_sb[:, 0:4, 0:128], wq_ch[:, 0:4, 0:128]
                    )
                    nc.sync.dma_start(
                        xT[:, 0:4, 0:256], xt_ch[:, 0:4, 0:256]
                    )
                    nc.sync.dma_start(
                        wq_sb[:, 4:8, 0:128], wq_ch[:, 4:8, 0:128]
                    )
                    nc.sync.dma_start(
                        xT[:, 4:8, 0:256], xt_ch[:, 4:8, 0:256]
                    )
                    nc.sync.dma_start(xT[:, :, 256:512], xt_ch[:, :, 256:512])
                    nc.sync.dma_start(
                        wq_sb[:, :, 128:512], wq_ch[:, :, 128:512]
                    )
                    nc.sync.dma_start(
                        wq_sb[:, :, 512:1024], wq_ch[:, :, 512:1024]
                    )
                    nc.sync.dma_start(wk_sb, chunked(wk_d))
                    nc.sync.dma_start(wv_sb, chunked(wv_d))
                    for tj in range(1, 4):
                        nc.sync.dma_start(
                            xT[:, :, tj * 512:(tj + 1) * 512],
                            xt_ch[:, :, tj * 512:(tj + 1) * 512],
                        )

                    # q^T[e, q] for this core's queries (x^T cols 0:512),
                    # written into the zero-padded layout. ej=0 runs in two
                    # column halves so it starts after just the first x^T
                    # DMA piece.
                    cp = nc.vector.tensor_copy
                    for sel in range(2):
                        qph = qpsum.tile([128, QH], F32, tag="qps")
                        for jd in range(DC):
                            nc.tensor.matmul(
                                qph,
                                lhsT=wq_sb[:, jd, 0:128],
                                rhs=xT[:, jd, sel * QH:(sel + 1) * QH],
                                start=(jd == 0),
                                stop=(jd == DC - 1),
                            )
                        cp(qpad[0:64, 0, sel, 0:QH], qph[0:64, :])
                        cp(qpad[64:128, 0, sel, QH:2 * QH], qph[64:128, :])
                    for ej in range(1, DC):
                        qps = qpsum.tile([128, 512], F32, tag="qps")
                        for jd in range(DC):
                            nc.tensor.matmul(
                                qps,
                                lhsT=wq_sb[:, jd, ej * 128:(ej + 1) * 128],
                                rhs=xT[:, jd, 0:512],
                                start=(jd == 0),
                                stop=(jd == DC - 1),
                            )
                        for sel in range(2):
                            cp(
                                qpad[0:64, ej, sel, 0:QH],
                                qps[0:64, sel * QH:(sel + 1) * QH],
                            )
                            cp(
                                qpad[64:128, ej, sel, QH:2 * QH],
                                qps[64:128, sel * QH:(sel + 1) * QH],
                            )

                # ---------------- pass A: qh=0 + K/V production ----------
                with (
                    tc.tile_pool(name="accA", bufs=1, space="PSUM") as accp,
                    tc.tile_pool(name="scA", bufs=1, space="PSUM") as scp,
                    tc.tile_pool(name="Ep", bufs=2) as Ep,
                    tc.tile_pool(name="smx", bufs=3) as smx,
                ):
                    acc0 = accp.tile([128, D], F32)
                    acc1 = accp.tile([128, D], F32)
                    accs = [acc0, acc1]
                    # column-split accumulation groups share PSUM banks;
                    # start=True clears beyond its own columns on this HW,
                    # so pre-zero the banks and accumulate with start=False.
                    nc.vector.memset(acc0, 0.0)
                    nc.vector.memset(acc1, 0.0)
                    pend = []  # softmax+PV closures, lagged one chunk
                    kvp_ctx = tc.tile_pool(name="kvps", bufs=2, space="PSUM")
                    kvp = kvp_ctx.__enter__()

                    def emit_k(tj, ej, dve_evac=False):
                        kps = kvp.tile([128, 512], F32, tag="kv")
                        for jd in range(DC):
                            nc.tensor.matmul(
                                kps,
                                lhsT=wk_sb[:, jd, ej * 128:(ej + 1) * 128],
                                rhs=xT[:, jd, tj * 512:(tj + 1) * 512],
                                start=(jd == 0),
                                stop=(jd == DC - 1),
                            )
                        # in the chunks that also host pre-groups, ACT is
                        # near-saturated and kvps slot reuse gates on these
                        # evacs — route them to DVE there
                        cp = (
                            nc.vector.tensor_copy if dve_evac
                            else nc.scalar.copy
                        )
                        cp(kT[:, ej, tj * 512:(tj + 1) * 512], kps)

                    def emit_v(kc, ehs=(0, 1)):
                        for eh in ehs:
                            vps = kvp.tile([128, 512], F32, tag="kv")
                            for jd in range(DC):
                                nc.tensor.matmul(
                                    vps,
                                    lhsT=xT[:, jd, kc * 128:(kc + 1) * 128],
                                    rhs=wv_sb[:, jd,
                                              eh * 512:(eh + 1) * 512],
                                    start=(jd == 0),
                                    stop=(jd == DC - 1),
                                )
                            cp = (
                                nc.vector.tensor_copy if eh == 0
                                else nc.scalar.copy
                            )
                            cp(v_sb[:, kc, eh * 512:(eh + 1) * 512], vps)

                    def emit_pre(ci, pr):
                        # one head-pair of qh1 scores for chunk 10+ci, exp'd
                        # straight into the persistent ppre tile. Uses a
                        # kvps PSUM slot so it never contends with the
                        # single-buffered qh0 scores tile.
                        kc_t = 10 + ci
                        sc = kvp.tile([128, 512], F32, tag="kv")
                        nc.tensor.matmul(
                            sc,
                            lhsT=kT[:, pr, kc_t * 128:(kc_t + 1) * 128],
                            rhs=qpad[:, pr, 1, :],
                            start=True,
                            stop=True,
                        )
                        nc.scalar.activation(
                            ppre[:, ci, 2 * pr:2 * pr + 2, :], sc, EXP,
                            scale=SCALE,
                        )

                    # prologue: k^T superstep 0 (keys 0:512) + v chunk 0
                    for ej in range(DC):
                        emit_k(0, ej)
                    emit_v(0)

                    for kc in range(TC - 2):
                        # K/V lookahead fillers, emitted BETWEEN score
                        # groups: the scores PSUM tile is single-buffered
                        # (bank budget), so group g+1's matmuls wait on
                        # group g's exp — the filler keeps the PE busy
                        # through that and through the softmax chain.
                        fillers = []
                        if kc < 12:
                            tj = kc // 4 + 1
                            dve = False
                            fillers.append(
                                lambda tj=tj, e=2 * (kc % 4), d=dve:
                                emit_k(tj, e, d)
                            )
                            fillers.append(
                                lambda tj=tj, e=2 * (kc % 4) + 1, d=dve:
                                emit_k(tj, e, d)
                            )
                        # V production compressed to kc<=13 so kvps can
                        # close before the last two chunks (which then run
                        # with double-buffered scores, see below)
                        if kc < 12:
                            fillers.append(
                                lambda kc=kc: emit_v(kc + 1, (0,))
                            )
                            fillers.append(
                                lambda kc=kc: emit_v(kc + 1, (1,))
                            )
                        elif kc == 12:
                            for args in ((13, (0,)), (13, (1,)),
                                         (14, (0,)), (14, (1,))):
                                fillers.append(
                                    lambda a=args: emit_v(*a)
                                )
                        elif kc == 13:
                            fillers.append(lambda: emit_v(15, (0,)))
                            fillers.append(lambda: emit_v(15, (1,)))
                        # qh1-precompute head-pairs as extra fillers, placed
                        # in the PE-heavy early-middle chunks. Pair (ci, pr)
                        # needs kT e-chunk pr of the tj=2 superstep, written
                        # at kc=4+pr//2 (possibly earlier in this same
                        # chunk's filler list — emission order covers it).
                        PRE_SCHED = {
                            5: [(0, 0), (0, 1), (0, 2)],
                            6: [(0, 3), (0, 4), (0, 5)],
                            7: [(0, 6), (0, 7), (1, 0)],
                            8: [(1, 1), (1, 2), (1, 3)],
                            9: [(1, 4), (1, 5), (1, 6)],
                            10: [(1, 7)],
                        }
                        for ci, pr in PRE_SCHED.get(kc, ()):
                            fillers.append(
                                lambda ci=ci, pr=pr: emit_pre(ci, pr)
                            )
                        # precompute softmax, spread so DVE never exceeds
                        # its per-chunk slack: head-sum tree in one chunk,
                        # the P multiply split over the next two
                        if kc == 8:
                            _smx_pre_tree(nc, smx, ppre, rpre, 0)
                        elif kc in (9, 10):
                            _smx_pre_mult(nc, ppre, rpre, 0, kc - 9)
                        elif kc == 11:
                            _smx_pre_tree(nc, smx, ppre, rpre, 1)
                        elif kc in (12, 13):
                            _smx_pre_mult(nc, ppre, rpre, 1, kc - 12)
                        if fillers:
                            fillers.pop(0)()
                        Et = _scores(nc, scp, Ep, kT, qpad, kc, qh=0,
                                     fillers=fillers)
                        pend.append(
                            lambda kc=kc, Et=Et: _softmax_pv(
                                nc, smx, v_sb, accs, Et, kc, dve_heads=13
                            )
                        )
                        if len(pend) > 1:
                            pend.pop(0)()
                    # kvps' banks free up here; kc=14/15 run with double-
                    # buffered scores so their exp round-trips don't stall
                    # the PE (no fillers remain to cover them).
                    kvp_ctx.__exit__(None, None, None)
                    with tc.tile_pool(
                        name="scA2", bufs=1, space="PSUM"
                    ) as scp2:
                        for kc in (14, 15):
                            Et = _scores(nc, (scp, scp2), Ep, kT, qpad, kc,
                                         qh=0)
                            pend.append(
                                lambda kc=kc, Et=Et: _softmax_pv(
                                    nc, smx, v_sb, accs, Et, kc,
                                    dve_heads=16 if kc == TC - 1 else 13
                                )
                            )
                            pend.pop(0)()
                        pend.pop(0)()
                        # per-half evac + xbar transpose, emitted as soon
                        # as each acc half's final PV lands
                        nc.scalar.copy(att[:, 0, :], acc0)
                        nc.sync.dma_start_transpose(
                            aT[:, :, 0:128], att[:, 0, :]
                        )
                        nc.vector.tensor_copy(att[:, 1, :], acc1)
                        nc.sync.dma_start_transpose(
                            aT[:, :, 128:256], att[:, 1, :]
                        )

            # ---------------- pass B: qh=1 ----------
            # PV runs in the attn^T[d, q] orientation, accumulating straight
            # into an [d, q] PSUM tile (aTacc): costs 2x the PE cycles of the
            # [q, d] orientation, but pass B's PE has slack (it is ACT/DVE
            # paced) and this removes the evac + xbar-transpose serial chain
            # from the tail.
            with tc.tile_pool(name="wpool", bufs=1) as wpp:
                # w_proj + bias loads deferred here: DMA is idle by now and
                # keeping them out of the pass-A SBUF footprint makes room
                # for ppre
                wp_sb = wpp.tile([128, DC, D], F16)
                bi_sb = wpp.tile([128, D], F32)
                nc.sync.dma_start(wp_sb, chunked(wp_d))
                nc.sync.dma_start(bi_sb, bias_d)

                with (
                    tc.tile_pool(name="accB", bufs=1, space="PSUM") as accpB,
                    tc.tile_pool(name="EpB", bufs=5) as EpB,
                    tc.tile_pool(name="smxB", bufs=4) as smxB,
                ):
                    # aTacc is created AFTER the first chunks' scores tiles
                    # so the allocator hands scB the PSUM banks that pass A
                    # frees first (its scores pools, free after the last
                    # exp) — pass B's first scores then never wait for the
                    # slower accA -> att-evac drain.
                    aTacc = None
                    pend = []  # PV lags three chunks: covers softmax chain
                    with tc.tile_pool(
                        name="scB", bufs=2, space="PSUM"
                    ) as scpB:
                        for kc in (0, 1, 2, 3, 4, 5, 6, 7, 8, 9,
                                   12, 13, 14, 15):
                            Et = _scores(nc, scpB, EpB, kT, qpad, kc, qh=1)
                            if aTacc is None:
                                aTacc = accpB.tile([128, DC, QH], F32)
                                # GPSIMD cannot access PSUM: DVE memsets
                                nc.vector.memset(aTacc[:, 0:4, :], 0.0)
                                nc.vector.memset(aTacc[:, 4:8, :], 0.0)
                            pend.append(
                                lambda kc=kc, Et=Et: _softmax_pv_direct(
                                    nc, smxB, v_sb, aTacc, Et, kc,
                                    dve_heads=12, stop=False,
                                )
                            )
                            if len(pend) > 3:
                                pend.pop(0)()
                    # scB's 4 banks are free now: the projection PSUM opens
                    # here so proj overlaps the pass-B drain and the PE
                    # never cools down (p-state) before the tail matmuls.
                    with (
                        tc.tile_pool(
                            name="prjps", bufs=4, space="PSUM"
                        ) as prjp,
                        tc.tile_pool(name="outp", bufs=4) as outp,
                    ):
                        # drain: PV for the precomputed chunks 10/11 has no
                        # softmax dependency — it fills the PE while chunks
                        # 13..15's softmax chains complete. Accumulation
                        # order is commutative; the stop flag rides on the
                        # last-emitted PV (chunk 11).
                        pend.pop(0)()                      # smx+PV chunk 13
                        _pv_pre(nc, aTacc, v_sb, ppre, 0, stop=False)
                        pend.pop(0)()                      # smx+PV chunk 14
                        pend.pop(0)()                      # smx+PV chunk 15
                        _pv_pre(nc, aTacc, v_sb, ppre, 1, stop=True)
                        # evacuate attn^T qh1 -> aT: two batched copies on
                        # ACT + DVE (GPSIMD cannot read PSUM), overlapped
                        # by the qh0 proj groups below
                        nc.scalar.copy(
                            aT[:, 0:4, 256:512], aTacc[:, 0:4, :]
                        )
                        nc.vector.tensor_copy(
                            aT[:, 4:8, 256:512], aTacc[:, 4:8, :]
                        )
                        # qh0's projection first: its aT half has been ready
                        # since pass A — no dependency on the evacs above
                        for qs in (0, 1, 2, 3):
                            for eh in range(2):
                                pm = prjp.tile([128, 512], F32, tag="pm")
                                for jd in range(DC):
                                    nc.tensor.matmul(
                                        pm,
                                        lhsT=aT[:, jd,
                                                qs * 128:(qs + 1) * 128],
                                        rhs=wp_sb[:, jd,
                                                  eh * 512:(eh + 1) * 512],
                                        start=(jd == 0),
                                        stop=(jd == DC - 1),
                                    )
                                ot = outp.tile([128, 512], F32, tag="ot")
                                nc.vector.tensor_tensor(
                                    ot, pm,
                                    bi_sb[:, eh * 512:(eh + 1) * 512],
                                    ADD,
                                )
                                nc.sync.dma_start(
                                    out_ch[:, qs,
                                           eh * 512:(eh + 1) * 512],
                                    ot,
                                )

    nc.compile()
    return nc


def _scores(nc, scp, Ep, kT, qpad, kc, qh, fillers=(), Et_out=None):
    """QK^T scores + fused scale/exp evacuation for one key chunk.

    `fillers` are emitted between score groups to give the PE independent
    work while the single-buffered scores tile round-trips through exp.
    `scp` may be a tuple of pools — groups then alternate between them.
    """
    scps = scp if isinstance(scp, tuple) else (scp,)
    fillers = list(fillers)
    Et = Et_out if Et_out is not None else Ep.tile([128, H, QH], F16, tag="E")
    for g in range(4):
        sc = scps[g % len(scps)].tile([128, 1024], F32, tag="sc")
        for i in range(2):
            pr = 2 * g + i
            nc.tensor.matmul(
                sc[:, i * 512:(i + 1) * 512],
                lhsT=kT[:, pr, kc * 128:(kc + 1) * 128],
                rhs=qpad[:, pr, qh, :],
                start=True,
                stop=True,
            )
        nc.scalar.activation(Et[:, 4 * g:4 * g + 4, :], sc, EXP, scale=SCALE)
        if fillers:
            fillers.pop(0)()
    while fillers:
        fillers.pop(0)()
    return Et


def _head_sum(nc, smx, Et, kc, r_out=None):
    """S = sum over heads, R = 1/S.

    The l1 level is split so the post-last-exp serial chain is short:
    tmp[i] = Et[i] + Et[i+8]. Pieces [0:2] (Pool) and [2:4] (DVE) only need
    exp groups g0 and g2, so they run while g3's exp is still in flight;
    only the [4:8] piece (needs g1 + g3) sits on the critical chain.
    """
    tmp = smx.tile([128, H // 2, QH], F16, tag="tmp")
    if kc == TC - 1:
        # final chunk: all on VectorE — the chain gates the pass drain and
        # Pool->DVE hops would lengthen it
        nc.vector.tensor_tensor(tmp[:, 0:4], Et[:, 0:4], Et[:, 8:12], ADD)
        nc.vector.tensor_tensor(tmp[:, 4:8], Et[:, 4:8], Et[:, 12:16], ADD)
    else:
        nc.gpsimd.tensor_tensor(tmp[:, 0:2], Et[:, 0:2], Et[:, 8:10], ADD)
        nc.vector.tensor_tensor(tmp[:, 2:4], Et[:, 2:4], Et[:, 10:12], ADD)
        nc.vector.tensor_tensor(tmp[:, 4:8], Et[:, 4:8], Et[:, 12:16], ADD)
    nc.vector.tensor_tensor(tmp[:, 0:4], tmp[:, 0:4], tmp[:, 4:8], ADD)
    nc.vector.tensor_tensor(tmp[:, 0:2], tmp[:, 0:2], tmp[:, 2:4], ADD)
    nc.vector.tensor_tensor(tmp[:, 0:1], tmp[:, 0:1], tmp[:, 1:2], ADD)
    if r_out is None:
        r_out = smx.tile([128, 1, QH], F16, tag="r")
    with nc.allow_low_precision(
        reason="softmax denominator reciprocal in fp16"
    ):
        nc.vector.reciprocal(r_out, tmp[:, 0:1])
    return r_out


def _smx_pre_tree(nc, smx, ppre, rpre, ci):
    """Head-sum + reciprocal for a precomputed qh1 chunk, 1/S -> rpre."""
    _head_sum(nc, smx, ppre[:, ci], kc=0, r_out=rpre[:, ci])


def _smx_pre_mult(nc, ppre, rpre, ci, half):
    """One half of P = E * (1/S) for a precomputed chunk, in place."""
    Et = ppre[:, ci]
    r = rpre[:, ci]
    a = half * 8
    nc.vector.tensor_tensor(
        Et[:, a:a + 6], Et[:, a:a + 6], r.to_broadcast([128, 6, QH]), MULT
    )
    nc.gpsimd.tensor_tensor(
        Et[:, a + 6:a + 8], Et[:, a + 6:a + 8],
        r.to_broadcast([128, 2, QH]), MULT,
    )


def _pv_pre(nc, aTacc, v_sb, ppre, ci, stop):
    """PV for a precomputed chunk (P already normalized in ppre). Emitted
    in the pass-B drain; the last-emitted call carries the accumulation-
    group stop."""
    kc = 10 + ci
    for h in range(H):
        pj = h // 2
        po = (h % 2) * 64
        nc.tensor.matmul(
            aTacc[po:po + 64, pj, :],
            lhsT=v_sb[:, kc, h * HD:(h + 1) * HD],
            rhs=ppre[:, ci, h, :],
            start=False,
            stop=stop,
            skip_group_check=True,
        )


def _softmax_pv(nc, smx, v_sb, accs, Et, kc, dve_heads):
    """Head-axis softmax + PV accumulation for one key chunk."""
    r = _head_sum(nc, smx, Et, kc)
    a = dve_heads
    nc.vector.tensor_tensor(
        Et[:, 0:a], Et[:, 0:a], r.to_broadcast([128, a, QH]), MULT
    )
    if a < H:
        nc.gpsimd.tensor_tensor(
            Et[:, a:H], Et[:, a:H], r.to_broadcast([128, H - a, QH]), MULT
        )
    # PV: attn[q, d] orientation, PSUM accumulation across all key chunks.
    # qc-outer so acc0 finishes a half-chunk early — its evacuation and
    # transpose at pass A's end start sooner.
    for qc in range(2):
        for h in range(H):
            nc.tensor.matmul(
                accs[qc][:, h * HD:(h + 1) * HD],
                lhsT=Et[:, h, qc * 128:(qc + 1) * 128],
                rhs=v_sb[:, kc, h * HD:(h + 1) * HD],
                start=False,
                stop=(kc == TC - 1),
                skip_group_check=True,
            )


def _softmax_pv_direct(nc, smx, v_sb, aTacc, Et, kc, dve_heads, stop):
    """Head-axis softmax + PV accumulation in the attn^T[d, q] orientation.

    Each head's PV emits one [64-partition, QH] matmul accumulating into the
    partition half of aTacc's d-chunk that holds that head's dims.
    """
    r = _head_sum(nc, smx, Et, kc)
    a = dve_heads
    nc.vector.tensor_tensor(
        Et[:, 0:a], Et[:, 0:a], r.to_broadcast([128, a, QH]), MULT
    )
    if a < H:
        nc.gpsimd.tensor_tensor(
            Et[:, a:H], Et[:, a:H], r.to_broadcast([128, H - a, QH]), MULT
        )
    for h in range(H):
        pj = h // 2
        po = (h % 2) * 64
        nc.tensor.matmul(
            aTacc[po:po + 64, pj, :],
            lhsT=v_sb[:, kc, h * HD:(h + 1) * HD],
            rhs=Et[:, h, :],
            start=False,
            stop=stop,
            skip_group_check=True,
        )


def get_nc():
    global _CACHED_NC
    if _CACHED_NC is None:
        _CACHED_NC = _build_nc()
    return _CACHED_NC


def kernel(x, w_qkv, w_proj, b_proj, _trace=False, _tmpdir=None):
    x = np.asarray(x, dtype=np.float32)
    w_qkv = np.asarray(w_qkv, dtype=np.float32)
    w_proj = np.asarray(w_proj, dtype=np.float32)
    b_proj = np.asarray(b_proj, dtype=np.float32)

    # Host-side layout prep: transpose + fp16 casts + per-core rotation.
    xT = [np.ascontiguousarray(x[b].T).astype(np.float16) for b in range(B)]
    wq = np.ascontiguousarray(w_qkv[:, 0:D]).astype(np.float16)
    wk = np.ascontiguousarray(w_qkv[:, D:2 * D]).astype(np.float16)
    wv = np.ascontiguousarray(w_qkv[:, 2 * D:3 * D]).astype(np.float16)
    wp = w_proj.astype(np.float16)
    bias = np.ascontiguousarray(
        np.broadcast_to(b_proj, (128, D))
    ).astype(np.float32)

    in_maps = []
    for c in range(NCORES):
        b = c // (NCORES // B)
        qofs = (c % (NCORES // B)) * QS
        xt_rot = np.ascontiguousarray(np.roll(xT[b], -qofs, axis=1))
        in_maps.append(
            {
                "xt": xt_rot,
                "wq": wq,
                "wk": wk,
                "wv": wv,
                "wp": wp,
                "bias": bias,
            }
        )

    nc = get_nc()
    res = bass_utils.run_bass_kernel_spmd(
        nc,
        in_maps,
        core_ids=list(range(NCORES)),
        trace=_trace,
        tmpdir=_tmpdir,
    )

    out = np.empty((B, T, D), dtype=np.float32)
    for c in range(NCORES):
        b = c // (NCORES // B)
        qofs = (c % (NCORES // B)) * QS
        out[b, qofs:qofs + QS] = res.results[c]["out"]
    if _trace:
        kernel._last_results = res
    return out



# revision 80
# speedup vs baseline: 1.0032x; 1.0032x over previous
"""Trainium2 Bass kernel for nn_Attention_46995532153449.

Module: qkv = x @ w_qkv; per-head scores = q k^T * hd^-0.5; softmax over the
HEAD axis (axis=1); attn = probs @ v; out = attn @ w_proj + b_proj.

Shapes: B=2, T=2048, D=1024, H=16, HD=64.

Sharding: data-parallel over (batch, query-block): core c handles batch c//4
and queries [(c%4)*512, (c%4+1)*512). The head-axis softmax is local (each
core holds all 16 heads for its query slice). K/V for the whole batch are
recomputed per core (collectives are priced far above their compute saving
by the cost model, so no cross-core exchange).

Structure (all chosen against the TimelineSim cost model):
  - host feeds x^T fp16 with columns ROTATED so the core's own 512 queries
    are columns 0:512 (one SPMD program, per-core data). Key order is a
    rotation, which attention is invariant to.
  - attention runs as two passes over the 16 key chunks (qh = 256-query
    halves) to fit PSUM. Pass A also produces K/V, software-pipelined as
    per-chunk lookahead filler emitted BETWEEN a chunk's score groups so
    the PE never stalls on the single-buffered scores tile's exp
    round-trips; PV lags one chunk.
  - pass A additionally PRECOMPUTES qh1's P (exp'd, normalized probs) for
    chunks 10/11 into a persistent tile: the score head-pairs ride the
    filler rotation through kvps PSUM slots (kc 5..9), the head-sum at
    kc=8/11 and the normalize-multiply halves at kc 9/10/12/13 land in
    DVE/Pool slack. Pass B then runs only 14 full chunks and its drain has
    dependency-free PV work.
  - pass A's last two chunks (no fillers left) run with double-buffered
    scores using kvps' freed banks, so they pace at ACT speed.
  - pass A PV uses the attn[q, d] orientation (half the PE cycles);
    pass B PV accumulates DIRECTLY in attn^T[d, q] ([64-partition, 256]
    matmuls into per-head partition halves): 2x the PE cycles, but pass B
    is ACT-paced with PE slack, and this deletes the evac+xbar-transpose
    serial chain before the output projection. PSUM accumulation across
    chunks uses pre-zeroed banks + start=False (column-split groups must
    not use start=True, which clears the whole partition row of a bank).
  - softmax head-sum: first tree level split [0:2] Pool / [2:4] DVE (both
    need only exp groups g0+g2) / [4:8] DVE (g1+g3) so the post-last-exp
    serial chain is short; then l2..l4 + reciprocal on DVE; P = E*R split
    DVE(13 pass A / 12 pass B)/GpSimd(rest). Pass B PV lags THREE chunks
    so the chain never stalls the in-order PE. GPSIMD cannot touch PSUM
    (verifier rule) — all PSUM-side copies/memsets are DVE/ACT.
  - pass B drain: PV for precomputed chunks 10/11 (stop flag on the last)
    covers chunks 14/15's softmax chains; the projection PSUM opens in
    scB's freed banks so proj runs straight off the drain with the PE
    p-state still hot; qh0's proj groups go first (their aT half is ready
    since pass A) covering the qh1 attn^T evacuation.
  - startup: DMA ring begins with quarter-size wq/xT pieces and Q's first
    e-chunk runs in column halves, so the PE starts ~3.5us in and streams.
    Per-DMA SP sequencer time is 565ns: more/smaller pieces starve the
    mid-Q stream (measured) — this split is the tuned balance.
  - pass A PV runs qc-outer so acc half 0 finishes early; each att half's
    evac + xbar transpose is emitted as soon as its half lands.

Measured: TimelineSim 272.9us/core (the harness metric), from 281.7us at
session start (382.1us original); hardware-run max rel err 6.7e-4 vs a
float64 reference.
Rejected avenues (measured): AllGather of K/V (cost model: 15us overhead +
40GB/s effective -> ~225us for 8MB, dwarfing the 82us of saved matmul);
remote_dma K/V exchange (unmodeled in no_exec TimelineSim -- the metric --
and RemoteDMA unsupported without MultiCoreSim); fp8-e4m3 DoubleRow scores
(empirically 2.8e-2 max rel err vs the 2e-2 gate); K/V-evac on DVE
(starves early-chunk DVE work); per-d-chunk attn^T evac copies (tile-level
write tracking serializes them -- batch into 2 wide copies instead).
"""

import numpy as np

import concourse.bacc as bacc
import concourse.mybir as mybir
import concourse.tile as tile
from concourse import bass_utils

B, T, D, H = 2, 2048, 1024, 16
HD = D // H          # 64
SCALE = HD ** -0.5   # 0.125
NCORES = 8
QS = B * T // NCORES  # 512 queries per core
QH = QS // 2          # 256-query halves (PSUM budget)
DC = D // 128         # 8 d/e chunks of 128
TC = T // 128         # 16 key chunks of 128

F16 = mybir.dt.float16
F32 = mybir.dt.float32
ADD = mybir.AluOpType.add
MULT = mybir.AluOpType.mult
EXP = mybir.ActivationFunctionType.Exp

_CACHED_NC = None


def _build_nc():
    nc = bacc.Bacc(
        "TRN2", target_bir_lowering=False, debug=False, enable_asserts=False
    )

    xt_d = nc.dram_tensor("xt", [D, T], F16, kind="ExternalInput").ap()
    wq_d = nc.dram_tensor("wq", [D, D], F16, kind="ExternalInput").ap()
    wk_d = nc.dram_tensor("wk", [D, D], F16, kind="ExternalInput").ap()
    wv_d = nc.dram_tensor("wv", [D, D], F16, kind="ExternalInput").ap()
    wp_d = nc.dram_tensor("wp", [D, D], F16, kind="ExternalInput").ap()
    bias_d = nc.dram_tensor("bias", [128, D], F32, kind="ExternalInput").ap()
    out_d = nc.dram_tensor("out", [QS, D], F32, kind="ExternalOutput").ap()

    def chunked(ap):  # [(c p), f] -> [p, c, f]
        return ap.rearrange("(c p) f -> p c f", p=128)

    xt_ch = chunked(xt_d)
    wq_ch = chunked(wq_d)
    out_ch = chunked(out_d)

    with tile.TileContext(nc) as tc:
        with tc.tile_pool(name="persist", bufs=1) as pp:
            kT = pp.tile([128, DC, T], F16)      # k^T: [e, t], e-chunk major
            v_sb = pp.tile([128, TC, D], F16)    # v: [t, e], t-chunk major
            # zero-padded q^T: for head pair pr and query half qh, columns
            # [0:QH] hold head 2pr's q^T at partitions 0:64 (zeros below),
            # columns [QH:2QH] hold head 2pr+1's at partitions 64:128, so
            # every scores matmul is a full-128-partition K=128 matmul.
            qpad = pp.tile([128, DC, 2, 2 * QH], F16)
            att = pp.tile([128, 2, D], F16)      # attn [q, d], per-qh reuse
            aT = pp.tile([128, DC, QS], F16)     # attn^T [d, q]
            # P (normalized probs) for qh1 chunks 10/11, precomputed in pass
            # A's ACT/DVE slack so pass B runs only 14 full chunks and its
            # drain has dependency-free PV work (pass B processes these
            # chunks' PV last — accumulation order is commutative)
            ppre = pp.tile([128, 2, H, QH], F16)
            rpre = pp.tile([128, 2, 1, QH], F16)  # 1/S for ppre chunks

            nc.gpsimd.memset(qpad, 0.0)

            with tc.tile_pool(name="pA", bufs=1) as pA:
                xT = pA.tile([128, DC, T], F16)
                wk_sb = pA.tile([128, DC, D], F16)
                wv_sb = pA.tile([128, DC, D], F16)

                with (
                    tc.tile_pool(name="qpool", bufs=1) as qp,
                    tc.tile_pool(name="qpsum", bufs=4, space="PSUM") as qpsum,
                ):
                    wq_sb = qp.tile([128, DC, D], F16)
                    # DMA ring order: first wq e-chunk -> own x^T in two
                    # pieces -> rest of wq -> wk -> wv -> remaining x^T
                    # pieces. Q's ej=0 starts after just wq0+xT-own-half;
                    # later ej's consume wq chunks as they stream in.
                    nc.sync.dma_start(
                        wq_sb[:, 0:4, 0:128], wq_ch[:, 0:4, 0:128]
                    )
                    nc.sync.dma_start(
                        xT[:, 0:4, 0:256], xt_ch[:, 0:4, 0:256]
                    )
                    nc.sync.dma_start(
                        wq_sb[:, 4:8, 0:128], wq_ch[:, 4:8, 0:128]
                    )
                    nc.sync.dma_start(
                        xT[:, 4:8, 0:256], xt_ch[:, 4:8, 0:256]
                    )
                    nc.sync.dma_start(xT[:, :, 256:512], xt_ch[:, :, 256:512])
                    nc.sync.dma_start(
                        wq_sb[:, :, 128:512], wq_ch[:, :, 128:512]
                    )
                    nc.sync.dma_start(
                        wq_sb[:, :, 512:1024], wq_ch[:, :, 512:1024]
                    )
                    nc.sync.dma_start(wk_sb, chunked(wk_d))
                    nc.sync.dma_start(wv_sb, chunked(wv_d))
                    for tj in range(1, 4):
                        nc.sync.dma_start(
                            xT[:, :, tj * 512:(tj + 1) * 512],
                            xt_ch[:, :, tj * 512:(tj + 1) * 512],
                        )

                    # q^T[e, q] for this core's queries (x^T cols 0:512),
                    # written into the zero-padded layout. ej=0 runs in two
                    # column halves so it starts after just the first x^T
                    # DMA piece.
                    cp = nc.vector.tensor_copy
                    for sel in range(2):
                        qph = qpsum.tile([128, QH], F32, tag="qps")
                        for jd in range(DC):
                            nc.tensor.matmul(
                                qph,
                                lhsT=wq_sb[:, jd, 0:128],
                                rhs=xT[:, jd, sel * QH:(sel + 1) * QH],
                                start=(jd == 0),
                                stop=(jd == DC - 1),
                            )
                        cp(qpad[0:64, 0, sel, 0:QH], qph[0:64, :])
                        cp(qpad[64:128, 0, sel, QH:2 * QH], qph[64:128, :])
                    for ej in range(1, DC):
                        qps = qpsum.tile([128, 512], F32, tag="qps")
                        for jd in range(DC):
                            nc.tensor.matmul(
                                qps,
                                lhsT=wq_sb[:, jd, ej * 128:(ej + 1) * 128],
                                rhs=xT[:, jd, 0:512],
                                start=(jd == 0),
                                stop=(jd == DC - 1),
                            )
                        for sel in range(2):
                            cp(
                                qpad[0:64, ej, sel, 0:QH],
                                qps[0:64, sel * QH:(sel + 1) * QH],
                            )
                            cp(
                                qpad[64:128, ej, sel, QH:2 * QH],
                                qps[64:128, sel * QH:(sel + 1) * QH],
                            )

                # ---------------- pass A: qh=0 + K/V production ----------
                with (
                    tc.tile_pool(name="accA", bufs=1, space="PSUM") as accp,
                    tc.tile_pool(name="scA", bufs=1, space="PSUM") as scp,
                    tc.tile_pool(name="Ep", bufs=2) as Ep,
                    tc.tile_pool(name="smx", bufs=3) as smx,
                ):
                    acc0 = accp.tile([128, D], F32)
                    acc1 = accp.tile([128, D], F32)
                    accs = [acc0, acc1]
                    # column-split accumulation groups share PSUM banks;
                    # start=True clears beyond its own columns on this HW,
                    # so pre-zero the banks and accumulate with start=False.
                    nc.vector.memset(acc0, 0.0)
                    nc.vector.memset(acc1, 0.0)
                    pend = []  # softmax+PV closures, lagged one chunk
                    kvp_ctx = tc.tile_pool(name="kvps", bufs=2, space="PSUM")
                    kvp = kvp_ctx.__enter__()

                    def emit_k(tj, ej, dve_evac=False):
                        kps = kvp.tile([128, 512], F32, tag="kv")
                        for jd in range(DC):
                            nc.tensor.matmul(
                                kps,
                                lhsT=wk_sb[:, jd, ej * 128:(ej + 1) * 128],
                                rhs=xT[:, jd, tj * 512:(tj + 1) * 512],
                                start=(jd == 0),
                                stop=(jd == DC - 1),
                            )
                        # in the chunks that also host pre-groups, ACT is
                        # near-saturated and kvps slot reuse gates on these
                        # evacs — route them to DVE there
                        cp = (
                            nc.vector.tensor_copy if dve_evac
                            else nc.scalar.copy
                        )
                        cp(kT[:, ej, tj * 512:(tj + 1) * 512], kps)

                    def emit_v(kc, ehs=(0, 1)):
                        for eh in ehs:
                            vps = kvp.tile([128, 512], F32, tag="kv")
                            for jd in range(DC):
                                nc.tensor.matmul(
                                    vps,
                                    lhsT=xT[:, jd, kc * 128:(kc + 1) * 128],
                                    rhs=wv_sb[:, jd,
                                              eh * 512:(eh + 1) * 512],
                                    start=(jd == 0),
                                    stop=(jd == DC - 1),
                                )
                            cp = (
                                nc.vector.tensor_copy if eh == 0
                                else nc.scalar.copy
                            )
                            cp(v_sb[:, kc, eh * 512:(eh + 1) * 512], vps)

                    def emit_pre(ci, pr):
                        # one head-pair of qh1 scores for chunk 10+ci, exp'd
                        # straight into the persistent ppre tile. Uses a
                        # kvps PSUM slot so it never contends with the
                        # single-buffered qh0 scores tile.
                        kc_t = 10 + ci
                        sc = kvp.tile([128, 512], F32, tag="kv")
                        nc.tensor.matmul(
                            sc,
                            lhsT=kT[:, pr, kc_t * 128:(kc_t + 1) * 128],
                            rhs=qpad[:, pr, 1, :],
                            start=True,
                            stop=True,
                        )
                        nc.scalar.activation(
                            ppre[:, ci, 2 * pr:2 * pr + 2, :], sc, EXP,
                            scale=SCALE,
                        )

                    # prologue: k^T superstep 0 (keys 0:512) + v chunk 0
                    for ej in range(DC):
                        emit_k(0, ej)
                    emit_v(0)

                    for kc in range(TC - 1):
                        # K/V lookahead fillers, emitted BETWEEN score
                        # groups: the scores PSUM tile is single-buffered
                        # (bank budget), so group g+1's matmuls wait on
                        # group g's exp — the filler keeps the PE busy
                        # through that and through the softmax chain.
                        fillers = []
                        if kc < 12:
                            tj = kc // 4 + 1
                            dve = False
                            fillers.append(
                                lambda tj=tj, e=2 * (kc % 4), d=dve:
                                emit_k(tj, e, d)
                            )
                            fillers.append(
                                lambda tj=tj, e=2 * (kc % 4) + 1, d=dve:
                                emit_k(tj, e, d)
                            )
                        # V(kc+1) produced just-in-time as fillers; the
                        # kc=14 pair absorbs the exp-wait holes there, and
                        # only kc=15 (V exhausted) needs the kvps-freed
                        # double-buffer below
                        if kc < TC - 1:
                            fillers.append(
                                lambda kc=kc: emit_v(kc + 1, (0,))
                            )
                            fillers.append(
                                lambda kc=kc: emit_v(kc + 1, (1,))
                            )
                        # qh1-precompute head-pairs as extra fillers, placed
                        # in the PE-heavy early-middle chunks. Pair (ci, pr)
                        # needs kT e-chunk pr of the tj=2 superstep, written
                        # at kc=4+pr//2 (possibly earlier in this same
                        # chunk's filler list — emission order covers it).
                        PRE_SCHED = {
                            5: [(0, 0), (0, 1), (0, 2)],
                            6: [(0, 3), (0, 4), (0, 5)],
                            7: [(0, 6), (0, 7), (1, 0)],
                            8: [(1, 1), (1, 2), (1, 3)],
                            9: [(1, 4), (1, 5), (1, 6)],
                            10: [(1, 7)],
                        }
                        for ci, pr in PRE_SCHED.get(kc, ()):
                            fillers.append(
                                lambda ci=ci, pr=pr: emit_pre(ci, pr)
                            )
                        # precompute softmax, spread so DVE never exceeds
                        # its per-chunk slack: head-sum tree in one chunk,
                        # the P multiply split over the next two
                        if kc == 8:
                            _smx_pre_tree(nc, smx, ppre, rpre, 0)
                        elif kc in (9, 10):
                            _smx_pre_mult(nc, ppre, rpre, 0, kc - 9)
                        elif kc == 11:
                            _smx_pre_tree(nc, smx, ppre, rpre, 1)
                        elif kc in (12, 13):
                            _smx_pre_mult(nc, ppre, rpre, 1, kc - 12)
                        if fillers:
                            fillers.pop(0)()
                        Et = _scores(nc, scp, Ep, kT, qpad, kc, qh=0,
                                     fillers=fillers)
                        pend.append(
                            lambda kc=kc, Et=Et: _softmax_pv(
                                nc, smx, v_sb, accs, Et, kc, dve_heads=13
                            )
                        )
                        if len(pend) > 1:
                            pend.pop(0)()
                    # kvps' banks free up here; kc=15 runs with double-
                    # buffered scores so its exp round-trips don't stall
                    # the PE (no fillers remain to cover them).
                    kvp_ctx.__exit__(None, None, None)
                    with tc.tile_pool(
                        name="scA2", bufs=1, space="PSUM"
                    ) as scp2:
                        for kc in (15,):
                            Et = _scores(nc, (scp, scp2), Ep, kT, qpad, kc,
                                         qh=0)
                            pend.append(
                                lambda kc=kc, Et=Et: _softmax_pv(
                                    nc, smx, v_sb, accs, Et, kc,
                                    dve_heads=16
                                )
                            )
                            pend.pop(0)()
                        pend.pop(0)()
                        # per-half evac + xbar transpose, emitted as soon
                        # as each acc half's final PV lands
                        nc.scalar.copy(att[:, 0, :], acc0)
                        nc.sync.dma_start_transpose(
                            aT[:, :, 0:128], att[:, 0, :]
                        )
                        nc.vector.tensor_copy(att[:, 1, :], acc1)
                        nc.sync.dma_start_transpose(
                            aT[:, :, 128:256], att[:, 1, :]
                        )

            # ---------------- pass B: qh=1 ----------
            # PV runs in the attn^T[d, q] orientation, accumulating straight
            # into an [d, q] PSUM tile (aTacc): costs 2x the PE cycles of the
            # [q, d] orientation, but pass B's PE has slack (it is ACT/DVE
            # paced) and this removes the evac + xbar-transpose serial chain
            # from the tail.
            with tc.tile_pool(name="wpool", bufs=1) as wpp:
                # w_proj + bias loads deferred here: DMA is idle by now and
                # keeping them out of the pass-A SBUF footprint makes room
                # for ppre
                wp_sb = wpp.tile([128, DC, D], F16)
                bi_sb = wpp.tile([128, D], F32)
                nc.sync.dma_start(wp_sb, chunked(wp_d))
                nc.sync.dma_start(bi_sb, bias_d)

                with (
                    tc.tile_pool(name="accB", bufs=1, space="PSUM") as accpB,
                    tc.tile_pool(name="EpB", bufs=5) as EpB,
                    tc.tile_pool(name="smxB", bufs=4) as smxB,
                ):
                    # aTacc is created AFTER the first chunks' scores tiles
                    # so the allocator hands scB the PSUM banks that pass A
                    # frees first (its scores pools, free after the last
                    # exp) — pass B's first scores then never wait for the
                    # slower accA -> att-evac drain.
                    aTacc = None
                    pend = []  # PV lags three chunks: covers softmax chain
                    with tc.tile_pool(
                        name="scB", bufs=2, space="PSUM"
                    ) as scpB:
                        for kc in (0, 1, 2, 3, 4, 5, 6, 7, 8, 9,
                                   12, 13, 14, 15):
                            Et = _scores(nc, scpB, EpB, kT, qpad, kc, qh=1)
                            if aTacc is None:
                                aTacc = accpB.tile([128, DC, QH], F32)
                                # GPSIMD cannot access PSUM: DVE memsets
                                nc.vector.memset(aTacc[:, 0:4, :], 0.0)
                                nc.vector.memset(aTacc[:, 4:8, :], 0.0)
                            pend.append(
                                lambda kc=kc, Et=Et: _softmax_pv_direct(
                                    nc, smxB, v_sb, aTacc, Et, kc,
                                    dve_heads=12, stop=False,
                                )
                            )
                            if len(pend) > 3:
                                pend.pop(0)()
                    # scB's 4 banks are free now: the projection PSUM opens
                    # here so proj overlaps the pass-B drain and the PE
                    # never cools down (p-state) before the tail matmuls.
                    with (
                        tc.tile_pool(
                            name="prjps", bufs=4, space="PSUM"
                        ) as prjp,
                        tc.tile_pool(name="outp", bufs=4) as outp,
                    ):
                        # drain: PV for the precomputed chunks 10/11 has no
                        # softmax dependency — it fills the PE while chunks
                        # 13..15's softmax chains complete. Accumulation
                        # order is commutative; the stop flag rides on the
                        # last-emitted PV (chunk 11).
                        pend.pop(0)()                      # smx+PV chunk 13
                        _pv_pre(nc, aTacc, v_sb, ppre, 0, stop=False)
                        pend.pop(0)()                      # smx+PV chunk 14
                        pend.pop(0)()                      # smx+PV chunk 15
                        _pv_pre(nc, aTacc, v_sb, ppre, 1, stop=True)
                        # evacuate attn^T qh1 -> aT: two batched copies on
                        # ACT + DVE (GPSIMD cannot read PSUM), overlapped
                        # by the qh0 proj groups below
                        nc.scalar.copy(
                            aT[:, 0:4, 256:512], aTacc[:, 0:4, :]
                        )
                        nc.vector.tensor_copy(
                            aT[:, 4:8, 256:512], aTacc[:, 4:8, :]
                        )
                        # qh0's projection first: its aT half has been ready
                        # since pass A — no dependency on the evacs above
                        for qs in (0, 1, 2, 3):
                            for eh in range(2):
                                pm = prjp.tile([128, 512], F32, tag="pm")
                                for jd in range(DC):
                                    nc.tensor.matmul(
                                        pm,
                                        lhsT=aT[:, jd,
                                                qs * 128:(qs + 1) * 128],
                                        rhs=wp_sb[:, jd,
                                                  eh * 512:(eh + 1) * 512],
                                        start=(jd == 0),
                                        stop=(jd == DC - 1),
                                    )
                                ot = outp.tile([128, 512], F32, tag="ot")
                                nc.vector.tensor_tensor(
                                    ot, pm,
                                    bi_sb[:, eh * 512:(eh + 1) * 512],
                                    ADD,
                                )
                                nc.sync.dma_start(
                                    out_ch[:, qs,
                                           eh * 512:(eh + 1) * 512],
                                    ot,
                                )

    nc.compile()
    return nc


def _scores(nc, scp, Ep, kT, qpad, kc, qh, fillers=(), Et_out=None):
    """QK^T scores + fused scale/exp evacuation for one key chunk.

    `fillers` are emitted between score groups to give the PE independent
    work while the single-buffered scores tile round-trips through exp.
    `scp` may be a tuple of pools — groups then alternate between them.
    """
    scps = scp if isinstance(scp, tuple) else (scp,)
    fillers = list(fillers)
    Et = Et_out if Et_out is not None else Ep.tile([128, H, QH], F16, tag="E")
    for g in range(4):
        sc = scps[g % len(scps)].tile([128, 1024], F32, tag="sc")
        for i in range(2):
            pr = 2 * g + i
            nc.tensor.matmul(
                sc[:, i * 512:(i + 1) * 512],
                lhsT=kT[:, pr, kc * 128:(kc + 1) * 128],
                rhs=qpad[:, pr, qh, :],
                start=True,
                stop=True,
            )
        nc.scalar.activation(Et[:, 4 * g:4 * g + 4, :], sc, EXP, scale=SCALE)
        if fillers:
            fillers.pop(0)()
    while fillers:
        fillers.pop(0)()
    return Et


def _head_sum(nc, smx, Et, kc, r_out=None):
    """S = sum over heads, R = 1/S.

    The l1 level is split so the post-last-exp serial chain is short:
    tmp[i] = Et[i] + Et[i+8]. Pieces [0:2] (Pool) and [2:4] (DVE) only need
    exp groups g0 and g2, so they run while g3's exp is still in flight;
    only the [4:8] piece (needs g1 + g3) sits on the critical chain.
    """
    tmp = smx.tile([128, H // 2, QH], F16, tag="tmp")
    if kc == TC - 1:
        # final chunk: all on VectorE — the chain gates the pass drain and
        # Pool->DVE hops would lengthen it
        nc.vector.tensor_tensor(tmp[:, 0:4], Et[:, 0:4], Et[:, 8:12], ADD)
        nc.vector.tensor_tensor(tmp[:, 4:8], Et[:, 4:8], Et[:, 12:16], ADD)
    else:
        nc.gpsimd.tensor_tensor(tmp[:, 0:2], Et[:, 0:2], Et[:, 8:10], ADD)
        nc.vector.tensor_tensor(tmp[:, 2:4], Et[:, 2:4], Et[:, 10:12], ADD)
        nc.vector.tensor_tensor(tmp[:, 4:8], Et[:, 4:8], Et[:, 12:16], ADD)
    nc.vector.tensor_tensor(tmp[:, 0:4], tmp[:, 0:4], tmp[:, 4:8], ADD)
    nc.vector.tensor_tensor(tmp[:, 0:2], tmp[:, 0:2], tmp[:, 2:4], ADD)
    nc.vector.tensor_tensor(tmp[:, 0:1], tmp[:, 0:1], tmp[:, 1:2], ADD)
    if r_out is None:
        r_out = smx.tile([128, 1, QH], F16, tag="r")
    with nc.allow_low_precision(
        reason="softmax denominator reciprocal in fp16"
    ):
        nc.vector.reciprocal(r_out, tmp[:, 0:1])
    return r_out


def _smx_pre_tree(nc, smx, ppre, rpre, ci):
    """Head-sum + reciprocal for a precomputed qh1 chunk, 1/S -> rpre."""
    _head_sum(nc, smx, ppre[:, ci], kc=0, r_out=rpre[:, ci])


def _smx_pre_mult(nc, ppre, rpre, ci, half):
    """One half of P = E * (1/S) for a precomputed chunk, in place."""
    Et = ppre[:, ci]
    r = rpre[:, ci]
    a = half * 8
    nc.vector.tensor_tensor(
        Et[:, a:a + 6], Et[:, a:a + 6], r.to_broadcast([128, 6, QH]), MULT
    )
    nc.gpsimd.tensor_tensor(
        Et[:, a + 6:a + 8], Et[:, a + 6:a + 8],
        r.to_broadcast([128, 2, QH]), MULT,
    )


def _pv_pre(nc, aTacc, v_sb, ppre, ci, stop):
    """PV for a precomputed chunk (P already normalized in ppre). Emitted
    in the pass-B drain; the last-emitted call carries the accumulation-
    group stop."""
    kc = 10 + ci
    for h in range(H):
        pj = h // 2
        po = (h % 2) * 64
        nc.tensor.matmul(
            aTacc[po:po + 64, pj, :],
            lhsT=v_sb[:, kc, h * HD:(h + 1) * HD],
            rhs=ppre[:, ci, h, :],
            start=False,
            stop=stop,
            skip_group_check=True,
        )


def _softmax_pv(nc, smx, v_sb, accs, Et, kc, dve_heads):
    """Head-axis softmax + PV accumulation for one key chunk."""
    r = _head_sum(nc, smx, Et, kc)
    a = dve_heads
    nc.vector.tensor_tensor(
        Et[:, 0:a], Et[:, 0:a], r.to_broadcast([128, a, QH]), MULT
    )
    if a < H:
        nc.gpsimd.tensor_tensor(
            Et[:, a:H], Et[:, a:H], r.to_broadcast([128, H - a, QH]), MULT
        )
    # PV: attn[q, d] orientation, PSUM accumulation across all key chunks.
    # qc-outer so acc0 finishes a half-chunk early — its evacuation and
    # transpose at pass A's end start sooner.
    for qc in range(2):
        for h in range(H):
            nc.tensor.matmul(
                accs[qc][:, h * HD:(h + 1) * HD],
                lhsT=Et[:, h, qc * 128:(qc + 1) * 128],
                rhs=v_sb[:, kc, h * HD:(h + 1) * HD],
                start=False,
                stop=(kc == TC - 1),
                skip_group_check=True,
            )


def _softmax_pv_direct(nc, smx, v_sb, aTacc, Et, kc, dve_heads, stop):
    """Head-axis softmax + PV accumulation in the attn^T[d, q] orientation.

    Each head's PV emits one [64-partition, QH] matmul accumulating into the
    partition half of aTacc's d-chunk that holds that head's dims.
    """
    r = _head_sum(nc, smx, Et, kc)
    a = dve_heads
    nc.vector.tensor_tensor(
        Et[:, 0:a], Et[:, 0:a], r.to_broadcast([128, a, QH]), MULT
    )
    if a < H:
        nc.gpsimd.tensor_tensor(
            Et[:, a:H], Et[:, a:H], r.to_broadcast([128, H - a, QH]), MULT
        )
    for h in range(H):
        pj = h // 2
        po = (h % 2) * 64
        nc.tensor.matmul(
            aTacc[po:po + 64, pj, :],
            lhsT=v_sb[:, kc, h * HD:(h + 1) * HD],
            rhs=Et[:, h, :],
            start=False,
            stop=stop,
            skip_group_check=True,
        )


def get_nc():
    global _CACHED_NC
    if _CACHED_NC is None:
        _CACHED_NC = _build_nc()
    return _CACHED_NC


def kernel(x, w_qkv, w_proj, b_proj, _trace=False, _tmpdir=None):
    x = np.asarray(x, dtype=np.float32)
    w_qkv = np.asarray(w_qkv, dtype=np.float32)
    w_proj = np.asarray(w_proj, dtype=np.float32)
    b_proj = np.asarray(b_proj, dtype=np.float32)

    # Host-side layout prep: transpose + fp16 casts + per-core rotation.
    xT = [np.ascontiguousarray(x[b].T).astype(np.float16) for b in range(B)]
    wq = np.ascontiguousarray(w_qkv[:, 0:D]).astype(np.float16)
    wk = np.ascontiguousarray(w_qkv[:, D:2 * D]).astype(np.float16)
    wv = np.ascontiguousarray(w_qkv[:, 2 * D:3 * D]).astype(np.float16)
    wp = w_proj.astype(np.float16)
    bias = np.ascontiguousarray(
        np.broadcast_to(b_proj, (128, D))
    ).astype(np.float32)

    in_maps = []
    for c in range(NCORES):
        b = c // (NCORES // B)
        qofs = (c % (NCORES // B)) * QS
        xt_rot = np.ascontiguousarray(np.roll(xT[b], -qofs, axis=1))
        in_maps.append(
            {
                "xt": xt_rot,
                "wq": wq,
                "wk": wk,
                "wv": wv,
                "wp": wp,
                "bias": bias,
            }
        )

    nc = get_nc()
    res = bass_utils.run_bass_kernel_spmd(
        nc,
        in_maps,
        core_ids=list(range(NCORES)),
        trace=_trace,
        tmpdir=_tmpdir,
    )

    out = np.empty((B, T, D), dtype=np.float32)
    for c in range(NCORES):
        b = c // (NCORES // B)
        qofs = (c % (NCORES // B)) * QS
        out[b, qofs:qofs + QS] = res.results[c]["out"]
    if _trace:
        kernel._last_results = res
    return out



# revision 90
# speedup vs baseline: 1.0076x; 1.0044x over previous
"""Trainium2 Bass kernel for nn_Attention_46995532153449.

Module: qkv = x @ w_qkv; per-head scores = q k^T * hd^-0.5; softmax over the
HEAD axis (axis=1); attn = probs @ v; out = attn @ w_proj + b_proj.

Shapes: B=2, T=2048, D=1024, H=16, HD=64.

Sharding: data-parallel over (batch, query-block): core c handles batch c//4
and queries [(c%4)*512, (c%4+1)*512). The head-axis softmax is local (each
core holds all 16 heads for its query slice). K/V for the whole batch are
recomputed per core (collectives are priced far above their compute saving
by the cost model, so no cross-core exchange).

Structure (all chosen against the TimelineSim cost model):
  - host feeds x^T fp16 with columns ROTATED so the core's own 512 queries
    are columns 0:512 (one SPMD program, per-core data). Key order is a
    rotation, which attention is invariant to.
  - attention runs as two passes over the 16 key chunks (qh = 256-query
    halves) to fit PSUM. Pass A also produces K/V, software-pipelined as
    per-chunk lookahead filler emitted BETWEEN a chunk's score groups so
    the PE never stalls on the single-buffered scores tile's exp
    round-trips; PV lags one chunk.
  - pass A additionally PRECOMPUTES qh1's P (exp'd, normalized probs) for
    chunks 10/11 into a persistent tile: the score head-pairs ride the
    filler rotation through kvps PSUM slots (kc 5..9), the head-sum at
    kc=8/11 and the normalize-multiply halves at kc 9/10/12/13 land in
    DVE/Pool slack. Pass B then runs only 14 full chunks and its drain has
    dependency-free PV work.
  - pass A's last two chunks (no fillers left) run with double-buffered
    scores using kvps' freed banks, so they pace at ACT speed.
  - pass A PV uses the attn[q, d] orientation (half the PE cycles);
    pass B PV accumulates DIRECTLY in attn^T[d, q] ([64-partition, 256]
    matmuls into per-head partition halves): 2x the PE cycles, but pass B
    is ACT-paced with PE slack, and this deletes the evac+xbar-transpose
    serial chain before the output projection. PSUM accumulation across
    chunks uses pre-zeroed banks + start=False (column-split groups must
    not use start=True, which clears the whole partition row of a bank).
  - softmax head-sum: first tree level split [0:2] Pool / [2:4] DVE (both
    need only exp groups g0+g2) / [4:8] DVE (g1+g3) so the post-last-exp
    serial chain is short; then l2..l4 + reciprocal on DVE; P = E*R split
    DVE(13 pass A / 12 pass B)/GpSimd(rest). Pass B PV lags THREE chunks
    so the chain never stalls the in-order PE. GPSIMD cannot touch PSUM
    (verifier rule) — all PSUM-side copies/memsets are DVE/ACT.
  - pass B drain: PV for precomputed chunks 10/11 (stop flag on the last)
    covers chunks 14/15's softmax chains; the projection PSUM opens in
    scB's freed banks so proj runs straight off the drain with the PE
    p-state still hot; qh0's proj groups go first (their aT half is ready
    since pass A) covering the qh1 attn^T evacuation.
  - startup: DMA ring begins with quarter-size wq/xT pieces and Q's first
    e-chunk runs in column halves, so the PE starts ~3.5us in and streams.
    Per-DMA SP sequencer time is 565ns: more/smaller pieces starve the
    mid-Q stream (measured) — this split is the tuned balance.
  - pass A PV runs qc-outer so acc half 0 finishes early; each att half's
    evac + xbar transpose is emitted as soon as its half lands.

Measured: TimelineSim 272.9us/core (the harness metric), from 281.7us at
session start (382.1us original); hardware-run max rel err 6.7e-4 vs a
float64 reference.
Rejected avenues (measured): AllGather of K/V (cost model: 15us overhead +
40GB/s effective -> ~225us for 8MB, dwarfing the 82us of saved matmul);
remote_dma K/V exchange (unmodeled in no_exec TimelineSim -- the metric --
and RemoteDMA unsupported without MultiCoreSim); fp8-e4m3 DoubleRow scores
(empirically 2.8e-2 max rel err vs the 2e-2 gate); K/V-evac on DVE
(starves early-chunk DVE work); per-d-chunk attn^T evac copies (tile-level
write tracking serializes them -- batch into 2 wide copies instead).
"""

import numpy as np

import concourse.bacc as bacc
import concourse.mybir as mybir
import concourse.tile as tile
from concourse import bass_utils

B, T, D, H = 2, 2048, 1024, 16
HD = D // H          # 64
SCALE = HD ** -0.5   # 0.125
NCORES = 8
QS = B * T // NCORES  # 512 queries per core
QH = QS // 2          # 256-query halves (PSUM budget)
DC = D // 128         # 8 d/e chunks of 128
TC = T // 128         # 16 key chunks of 128

F16 = mybir.dt.float16
F32 = mybir.dt.float32
ADD = mybir.AluOpType.add
MULT = mybir.AluOpType.mult
EXP = mybir.ActivationFunctionType.Exp

_CACHED_NC = None


def _build_nc():
    nc = bacc.Bacc(
        "TRN2", target_bir_lowering=False, debug=False, enable_asserts=False
    )

    xt_d = nc.dram_tensor("xt", [D, T], F16, kind="ExternalInput").ap()
    wq_d = nc.dram_tensor("wq", [D, D], F16, kind="ExternalInput").ap()
    wk_d = nc.dram_tensor("wk", [D, D], F16, kind="ExternalInput").ap()
    wv_d = nc.dram_tensor("wv", [D, D], F16, kind="ExternalInput").ap()
    wp_d = nc.dram_tensor("wp", [D, D], F16, kind="ExternalInput").ap()
    bias_d = nc.dram_tensor("bias", [128, D], F32, kind="ExternalInput").ap()
    out_d = nc.dram_tensor("out", [QS, D], F32, kind="ExternalOutput").ap()

    def chunked(ap):  # [(c p), f] -> [p, c, f]
        return ap.rearrange("(c p) f -> p c f", p=128)

    xt_ch = chunked(xt_d)
    wq_ch = chunked(wq_d)
    out_ch = chunked(out_d)

    with tile.TileContext(nc) as tc:
        with tc.tile_pool(name="persist", bufs=1) as pp:
            kT = pp.tile([128, DC, T], F16)      # k^T: [e, t], e-chunk major
            v_sb = pp.tile([128, TC, D], F16)    # v: [t, e], t-chunk major
            # zero-padded q^T: for head pair pr and query half qh, columns
            # [0:QH] hold head 2pr's q^T at partitions 0:64 (zeros below),
            # columns [QH:2QH] hold head 2pr+1's at partitions 64:128, so
            # every scores matmul is a full-128-partition K=128 matmul.
            qpad = pp.tile([128, DC, 2, 2 * QH], F16)
            att = pp.tile([128, 2, D], F16)      # attn [q, d], per-qh reuse
            aT = pp.tile([128, DC, QS], F16)     # attn^T [d, q]
            # P (normalized probs) for qh1 chunks 10/11, precomputed in pass
            # A's ACT/DVE slack so pass B runs only 14 full chunks and its
            # drain has dependency-free PV work (pass B processes these
            # chunks' PV last — accumulation order is commutative)
            ppre = pp.tile([128, 2, H, QH], F16)
            rpre = pp.tile([128, 2, 1, QH], F16)  # 1/S for ppre chunks

            nc.gpsimd.memset(qpad, 0.0)

            with tc.tile_pool(name="pA", bufs=1) as pA:
                xT = pA.tile([128, DC, T], F16)
                wk_sb = pA.tile([128, DC, D], F16)
                wv_sb = pA.tile([128, DC, D], F16)

                with (
                    tc.tile_pool(name="qpool", bufs=1) as qp,
                    tc.tile_pool(name="qpsum", bufs=4, space="PSUM") as qpsum,
                ):
                    wq_sb = qp.tile([128, DC, D], F16)
                    # DMA ring order: first wq e-chunk -> own x^T in two
                    # pieces -> rest of wq -> wk -> wv -> remaining x^T
                    # pieces. Q's ej=0 starts after just wq0+xT-own-half;
                    # later ej's consume wq chunks as they stream in.
                    nc.sync.dma_start(
                        wq_sb[:, 0:4, 0:128], wq_ch[:, 0:4, 0:128]
                    )
                    nc.sync.dma_start(
                        xT[:, 0:4, 0:256], xt_ch[:, 0:4, 0:256]
                    )
                    nc.sync.dma_start(
                        wq_sb[:, 4:8, 0:128], wq_ch[:, 4:8, 0:128]
                    )
                    nc.sync.dma_start(
                        xT[:, 4:8, 0:256], xt_ch[:, 4:8, 0:256]
                    )
                    nc.sync.dma_start(xT[:, :, 256:512], xt_ch[:, :, 256:512])
                    nc.sync.dma_start(
                        wq_sb[:, :, 128:512], wq_ch[:, :, 128:512]
                    )
                    nc.sync.dma_start(
                        wq_sb[:, :, 512:1024], wq_ch[:, :, 512:1024]
                    )
                    nc.sync.dma_start(wk_sb, chunked(wk_d))
                    nc.sync.dma_start(wv_sb, chunked(wv_d))
                    for tj in range(1, 4):
                        nc.sync.dma_start(
                            xT[:, :, tj * 512:(tj + 1) * 512],
                            xt_ch[:, :, tj * 512:(tj + 1) * 512],
                        )

                    # q^T[e, q] for this core's queries (x^T cols 0:512),
                    # written into the zero-padded layout. ej=0 runs in two
                    # column halves so it starts after just the first x^T
                    # DMA piece.
                    cp = nc.vector.tensor_copy
                    for sel in range(2):
                        qph = qpsum.tile([128, QH], F32, tag="qps")
                        for jd in range(DC):
                            nc.tensor.matmul(
                                qph,
                                lhsT=wq_sb[:, jd, 0:128],
                                rhs=xT[:, jd, sel * QH:(sel + 1) * QH],
                                start=(jd == 0),
                                stop=(jd == DC - 1),
                            )
                        cp(qpad[0:64, 0, sel, 0:QH], qph[0:64, :])
                        cp(qpad[64:128, 0, sel, QH:2 * QH], qph[64:128, :])
                    for ej in range(1, DC):
                        qps = qpsum.tile([128, 512], F32, tag="qps")
                        for jd in range(DC):
                            nc.tensor.matmul(
                                qps,
                                lhsT=wq_sb[:, jd, ej * 128:(ej + 1) * 128],
                                rhs=xT[:, jd, 0:512],
                                start=(jd == 0),
                                stop=(jd == DC - 1),
                            )
                        for sel in range(2):
                            cp(
                                qpad[0:64, ej, sel, 0:QH],
                                qps[0:64, sel * QH:(sel + 1) * QH],
                            )
                            cp(
                                qpad[64:128, ej, sel, QH:2 * QH],
                                qps[64:128, sel * QH:(sel + 1) * QH],
                            )

                # ---------------- pass A: qh=0 + K/V production ----------
                with (
                    tc.tile_pool(name="accA", bufs=1, space="PSUM") as accp,
                    tc.tile_pool(name="scA", bufs=1, space="PSUM") as scp,
                    tc.tile_pool(name="Ep", bufs=2) as Ep,
                    tc.tile_pool(name="smx", bufs=3) as smx,
                ):
                    acc0 = accp.tile([128, D], F32)
                    acc1 = accp.tile([128, D], F32)
                    accs = [acc0, acc1]
                    # column-split accumulation groups share PSUM banks;
                    # start=True clears beyond its own columns on this HW,
                    # so pre-zero the banks and accumulate with start=False.
                    nc.vector.memset(acc0, 0.0)
                    nc.vector.memset(acc1, 0.0)
                    pend = []  # softmax+PV closures, lagged one chunk
                    kvp_ctx = tc.tile_pool(name="kvps", bufs=2, space="PSUM")
                    kvp = kvp_ctx.__enter__()

                    def emit_k(tj, ej, dve_evac=False):
                        kps = kvp.tile([128, 512], F32, tag="kv")
                        for jd in range(DC):
                            nc.tensor.matmul(
                                kps,
                                lhsT=wk_sb[:, jd, ej * 128:(ej + 1) * 128],
                                rhs=xT[:, jd, tj * 512:(tj + 1) * 512],
                                start=(jd == 0),
                                stop=(jd == DC - 1),
                            )
                        # in the chunks that also host pre-groups, ACT is
                        # near-saturated and kvps slot reuse gates on these
                        # evacs — route them to DVE there
                        cp = (
                            nc.vector.tensor_copy if dve_evac
                            else nc.scalar.copy
                        )
                        cp(kT[:, ej, tj * 512:(tj + 1) * 512], kps)

                    def emit_v(kc, ehs=(0, 1)):
                        for eh in ehs:
                            vps = kvp.tile([128, 512], F32, tag="kv")
                            for jd in range(DC):
                                nc.tensor.matmul(
                                    vps,
                                    lhsT=xT[:, jd, kc * 128:(kc + 1) * 128],
                                    rhs=wv_sb[:, jd,
                                              eh * 512:(eh + 1) * 512],
                                    start=(jd == 0),
                                    stop=(jd == DC - 1),
                                )
                            cp = (
                                nc.vector.tensor_copy if eh == 0
                                else nc.scalar.copy
                            )
                            cp(v_sb[:, kc, eh * 512:(eh + 1) * 512], vps)

                    def emit_pre(ci, pr):
                        # one head-pair of qh1 scores for chunk 10+ci, exp'd
                        # straight into the persistent ppre tile. Uses a
                        # kvps PSUM slot so it never contends with the
                        # single-buffered qh0 scores tile.
                        kc_t = 10 + ci
                        sc = kvp.tile([128, 512], F32, tag="kv")
                        nc.tensor.matmul(
                            sc,
                            lhsT=kT[:, pr, kc_t * 128:(kc_t + 1) * 128],
                            rhs=qpad[:, pr, 1, :],
                            start=True,
                            stop=True,
                        )
                        nc.scalar.activation(
                            ppre[:, ci, 2 * pr:2 * pr + 2, :], sc, EXP,
                            scale=SCALE,
                        )

                    # prologue: k^T superstep 0 (keys 0:512) + v chunk 0
                    for ej in range(DC):
                        emit_k(0, ej)
                    emit_v(0)

                    for kc in range(TC - 1):
                        # K/V lookahead fillers, emitted BETWEEN score
                        # groups: the scores PSUM tile is single-buffered
                        # (bank budget), so group g+1's matmuls wait on
                        # group g's exp — the filler keeps the PE busy
                        # through that and through the softmax chain.
                        fillers = []
                        if kc < 12:
                            tj = kc // 4 + 1
                            dve = False
                            fillers.append(
                                lambda tj=tj, e=2 * (kc % 4), d=dve:
                                emit_k(tj, e, d)
                            )
                            fillers.append(
                                lambda tj=tj, e=2 * (kc % 4) + 1, d=dve:
                                emit_k(tj, e, d)
                            )
                        # V(kc+1) produced just-in-time as fillers; the
                        # kc=14 pair absorbs the exp-wait holes there, and
                        # only kc=15 (V exhausted) needs the kvps-freed
                        # double-buffer below
                        if kc < TC - 1:
                            fillers.append(
                                lambda kc=kc: emit_v(kc + 1, (0,))
                            )
                            fillers.append(
                                lambda kc=kc: emit_v(kc + 1, (1,))
                            )
                        # qh1-precompute head-pairs as extra fillers, placed
                        # in the PE-heavy early-middle chunks. Pair (ci, pr)
                        # needs kT e-chunk pr of the tj=2 superstep, written
                        # at kc=4+pr//2 (possibly earlier in this same
                        # chunk's filler list — emission order covers it).
                        PRE_SCHED = {
                            5: [(0, 0), (0, 1), (0, 2)],
                            6: [(0, 3), (0, 4), (0, 5)],
                            7: [(0, 6), (0, 7), (1, 0)],
                            8: [(1, 1), (1, 2), (1, 3)],
                            9: [(1, 4), (1, 5), (1, 6)],
                            10: [(1, 7)],
                        }
                        for ci, pr in PRE_SCHED.get(kc, ()):
                            fillers.append(
                                lambda ci=ci, pr=pr: emit_pre(ci, pr)
                            )
                        # precompute softmax, spread so DVE never exceeds
                        # its per-chunk slack: head-sum tree in one chunk,
                        # the P multiply split over the next two
                        if kc == 8:
                            _smx_pre_tree(nc, smx, ppre, rpre, 0)
                        elif kc in (9, 10):
                            _smx_pre_mult(nc, ppre, rpre, 0, kc - 9)
                        elif kc == 11:
                            _smx_pre_tree(nc, smx, ppre, rpre, 1)
                        elif kc in (12, 13):
                            _smx_pre_mult(nc, ppre, rpre, 1, kc - 12)
                        if fillers:
                            fillers.pop(0)()
                        Et = _scores(nc, scp, Ep, kT, qpad, kc, qh=0,
                                     fillers=fillers)
                        pend.append(
                            lambda kc=kc, Et=Et: _softmax_pv(
                                nc, smx, v_sb, accs, Et, kc, dve_heads=13
                            )
                        )
                        if len(pend) > 1:
                            pend.pop(0)()
                    # kvps' banks free up here; kc=15 runs with double-
                    # buffered scores so its exp round-trips don't stall
                    # the PE (no fillers remain to cover them).
                    kvp_ctx.__exit__(None, None, None)
                    with tc.tile_pool(
                        name="scA2", bufs=1, space="PSUM"
                    ) as scp2:
                        for kc in (15,):
                            Et = _scores(nc, (scp, scp2), Ep, kT, qpad, kc,
                                         qh=0)
                            pend.append(
                                lambda kc=kc, Et=Et: _softmax_pv(
                                    nc, smx, v_sb, accs, Et, kc,
                                    dve_heads=12
                                )
                            )
                            pend.pop(0)()
                        pend.pop(0)()
                        # per-half evac + xbar transpose, emitted as soon
                        # as each acc half's final PV lands
                        nc.scalar.copy(att[:, 0, :], acc0)
                        nc.sync.dma_start_transpose(
                            aT[:, :, 0:128], att[:, 0, :]
                        )
                        nc.vector.tensor_copy(att[:, 1, :], acc1)
                        nc.sync.dma_start_transpose(
                            aT[:, :, 128:256], att[:, 1, :]
                        )

            # ---------------- pass B: qh=1 ----------
            # PV runs in the attn^T[d, q] orientation, accumulating straight
            # into an [d, q] PSUM tile (aTacc): costs 2x the PE cycles of the
            # [q, d] orientation, but pass B's PE has slack (it is ACT/DVE
            # paced) and this removes the evac + xbar-transpose serial chain
            # from the tail.
            with tc.tile_pool(name="wpool", bufs=1) as wpp:
                # w_proj + bias loads deferred here: DMA is idle by now and
                # keeping them out of the pass-A SBUF footprint makes room
                # for ppre
                wp_sb = wpp.tile([128, DC, D], F16)
                bi_sb = wpp.tile([128, D], F32)
                nc.sync.dma_start(wp_sb, chunked(wp_d))
                nc.sync.dma_start(bi_sb, bias_d)

                with (
                    tc.tile_pool(name="accB", bufs=1, space="PSUM") as accpB,
                    tc.tile_pool(name="EpB", bufs=5) as EpB,
                    tc.tile_pool(name="smxB", bufs=4) as smxB,
                ):
                    # aTacc is created AFTER the first chunks' scores tiles
                    # so the allocator hands scB the PSUM banks that pass A
                    # frees first (its scores pools, free after the last
                    # exp) — pass B's first scores then never wait for the
                    # slower accA -> att-evac drain.
                    aTacc = None
                    pend = []  # PV lags three chunks: covers softmax chain
                    with tc.tile_pool(
                        name="scB", bufs=2, space="PSUM"
                    ) as scpB:
                        for kc in (0, 1, 2, 3, 4, 5, 6, 7, 8, 9,
                                   12, 13, 14, 15):
                            Et = _scores(nc, scpB, EpB, kT, qpad, kc, qh=1)
                            if aTacc is None:
                                aTacc = accpB.tile([128, DC, QH], F32)
                                # GPSIMD cannot access PSUM: DVE memsets
                                nc.vector.memset(aTacc[:, 0:4, :], 0.0)
                                nc.vector.memset(aTacc[:, 4:8, :], 0.0)
                            pend.append(
                                lambda kc=kc, Et=Et: _softmax_pv_direct(
                                    nc, smxB, v_sb, aTacc, Et, kc,
                                    dve_heads=12, stop=False,
                                )
                            )
                            if len(pend) > 3:
                                pend.pop(0)()
                    # scB's 4 banks are free now: the projection PSUM opens
                    # here so proj overlaps the pass-B drain and the PE
                    # never cools down (p-state) before the tail matmuls.
                    with (
                        tc.tile_pool(
                            name="prjps", bufs=4, space="PSUM"
                        ) as prjp,
                        tc.tile_pool(name="outp", bufs=4) as outp,
                    ):
                        # drain: PV for the precomputed chunks 10/11 has no
                        # softmax dependency — it fills the PE while chunks
                        # 13..15's softmax chains complete. Accumulation
                        # order is commutative; the stop flag rides on the
                        # last-emitted PV (chunk 11).
                        pend.pop(0)()                      # smx+PV chunk 13
                        _pv_pre(nc, aTacc, v_sb, ppre, 0, stop=False)
                        pend.pop(0)()                      # smx+PV chunk 14
                        pend.pop(0)()                      # smx+PV chunk 15
                        _pv_pre(nc, aTacc, v_sb, ppre, 1, stop=True)
                        # evacuate attn^T qh1 -> aT: two batched copies on
                        # ACT + DVE (GPSIMD cannot read PSUM), overlapped
                        # by the qh0 proj groups below
                        nc.scalar.copy(
                            aT[:, 0:4, 256:512], aTacc[:, 0:4, :]
                        )
                        nc.vector.tensor_copy(
                            aT[:, 4:8, 256:512], aTacc[:, 4:8, :]
                        )
                        # qh0's projection first: its aT half has been ready
                        # since pass A — no dependency on the evacs above
                        for qs in (0, 1, 2, 3):
                            for eh in range(2):
                                pm = prjp.tile([128, 512], F32, tag="pm")
                                for jd in range(DC):
                                    nc.tensor.matmul(
                                        pm,
                                        lhsT=aT[:, jd,
                                                qs * 128:(qs + 1) * 128],
                                        rhs=wp_sb[:, jd,
                                                  eh * 512:(eh + 1) * 512],
                                        start=(jd == 0),
                                        stop=(jd == DC - 1),
                                    )
                                ot = outp.tile([128, 512], F32, tag="ot")
                                nc.vector.tensor_tensor(
                                    ot, pm,
                                    bi_sb[:, eh * 512:(eh + 1) * 512],
                                    ADD,
                                )
                                nc.sync.dma_start(
                                    out_ch[:, qs,
                                           eh * 512:(eh + 1) * 512],
                                    ot,
                                )

    nc.compile()
    return nc


def _scores(nc, scp, Ep, kT, qpad, kc, qh, fillers=(), Et_out=None):
    """QK^T scores + fused scale/exp evacuation for one key chunk.

    `fillers` are emitted between score groups to give the PE independent
    work while the single-buffered scores tile round-trips through exp.
    `scp` may be a tuple of pools — groups then alternate between them.
    """
    scps = scp if isinstance(scp, tuple) else (scp,)
    fillers = list(fillers)
    Et = Et_out if Et_out is not None else Ep.tile([128, H, QH], F16, tag="E")
    for g in range(4):
        sc = scps[g % len(scps)].tile([128, 1024], F32, tag="sc")
        for i in range(2):
            pr = 2 * g + i
            nc.tensor.matmul(
                sc[:, i * 512:(i + 1) * 512],
                lhsT=kT[:, pr, kc * 128:(kc + 1) * 128],
                rhs=qpad[:, pr, qh, :],
                start=True,
                stop=True,
            )
        nc.scalar.activation(Et[:, 4 * g:4 * g + 4, :], sc, EXP, scale=SCALE)
        if fillers:
            fillers.pop(0)()
    while fillers:
        fillers.pop(0)()
    return Et


def _head_sum(nc, smx, Et, kc, r_out=None):
    """S = sum over heads, R = 1/S.

    The l1 level is split so the post-last-exp serial chain is short:
    tmp[i] = Et[i] + Et[i+8]. Pieces [0:2] (Pool) and [2:4] (DVE) only need
    exp groups g0 and g2, so they run while g3's exp is still in flight;
    only the [4:8] piece (needs g1 + g3) sits on the critical chain.
    """
    tmp = smx.tile([128, H // 2, QH], F16, tag="tmp")
    nc.gpsimd.tensor_tensor(tmp[:, 0:2], Et[:, 0:2], Et[:, 8:10], ADD)
    nc.vector.tensor_tensor(tmp[:, 2:4], Et[:, 2:4], Et[:, 10:12], ADD)
    nc.vector.tensor_tensor(tmp[:, 4:8], Et[:, 4:8], Et[:, 12:16], ADD)
    nc.vector.tensor_tensor(tmp[:, 0:4], tmp[:, 0:4], tmp[:, 4:8], ADD)
    nc.vector.tensor_tensor(tmp[:, 0:2], tmp[:, 0:2], tmp[:, 2:4], ADD)
    nc.vector.tensor_tensor(tmp[:, 0:1], tmp[:, 0:1], tmp[:, 1:2], ADD)
    if r_out is None:
        r_out = smx.tile([128, 1, QH], F16, tag="r")
    with nc.allow_low_precision(
        reason="softmax denominator reciprocal in fp16"
    ):
        nc.vector.reciprocal(r_out, tmp[:, 0:1])
    return r_out


def _smx_pre_tree(nc, smx, ppre, rpre, ci):
    """Head-sum + reciprocal for a precomputed qh1 chunk, 1/S -> rpre."""
    _head_sum(nc, smx, ppre[:, ci], kc=0, r_out=rpre[:, ci])


def _smx_pre_mult(nc, ppre, rpre, ci, half):
    """One half of P = E * (1/S) for a precomputed chunk, in place."""
    Et = ppre[:, ci]
    r = rpre[:, ci]
    a = half * 8
    nc.vector.tensor_tensor(
        Et[:, a:a + 6], Et[:, a:a + 6], r.to_broadcast([128, 6, QH]), MULT
    )
    nc.gpsimd.tensor_tensor(
        Et[:, a + 6:a + 8], Et[:, a + 6:a + 8],
        r.to_broadcast([128, 2, QH]), MULT,
    )


def _pv_pre(nc, aTacc, v_sb, ppre, ci, stop):
    """PV for a precomputed chunk (P already normalized in ppre). Emitted
    in the pass-B drain; the last-emitted call carries the accumulation-
    group stop."""
    kc = 10 + ci
    for h in range(H):
        pj = h // 2
        po = (h % 2) * 64
        nc.tensor.matmul(
            aTacc[po:po + 64, pj, :],
            lhsT=v_sb[:, kc, h * HD:(h + 1) * HD],
            rhs=ppre[:, ci, h, :],
            start=False,
            stop=stop,
            skip_group_check=True,
        )


def _softmax_pv(nc, smx, v_sb, accs, Et, kc, dve_heads):
    """Head-axis softmax + PV accumulation for one key chunk."""
    r = _head_sum(nc, smx, Et, kc)
    a = dve_heads
    nc.vector.tensor_tensor(
        Et[:, 0:a], Et[:, 0:a], r.to_broadcast([128, a, QH]), MULT
    )
    if a < H:
        nc.gpsimd.tensor_tensor(
            Et[:, a:H], Et[:, a:H], r.to_broadcast([128, H - a, QH]), MULT
        )
    # PV: attn[q, d] orientation, PSUM accumulation across all key chunks.
    # qc-outer so acc0 finishes a half-chunk early — its evacuation and
    # transpose at pass A's end start sooner.
    for qc in range(2):
        for h in range(H):
            nc.tensor.matmul(
                accs[qc][:, h * HD:(h + 1) * HD],
                lhsT=Et[:, h, qc * 128:(qc + 1) * 128],
                rhs=v_sb[:, kc, h * HD:(h + 1) * HD],
                start=False,
                stop=(kc == TC - 1),
                skip_group_check=True,
            )


def _softmax_pv_direct(nc, smx, v_sb, aTacc, Et, kc, dve_heads, stop):
    """Head-axis softmax + PV accumulation in the attn^T[d, q] orientation.

    Each head's PV emits one [64-partition, QH] matmul accumulating into the
    partition half of aTacc's d-chunk that holds that head's dims.
    """
    r = _head_sum(nc, smx, Et, kc)
    a = dve_heads
    nc.vector.tensor_tensor(
        Et[:, 0:a], Et[:, 0:a], r.to_broadcast([128, a, QH]), MULT
    )
    if a < H:
        nc.gpsimd.tensor_tensor(
            Et[:, a:H], Et[:, a:H], r.to_broadcast([128, H - a, QH]), MULT
        )
    for h in range(H):
        pj = h // 2
        po = (h % 2) * 64
        nc.tensor.matmul(
            aTacc[po:po + 64, pj, :],
            lhsT=v_sb[:, kc, h * HD:(h + 1) * HD],
            rhs=Et[:, h, :],
            start=False,
            stop=stop,
            skip_group_check=True,
        )


def get_nc():
    global _CACHED_NC
    if _CACHED_NC is None:
        _CACHED_NC = _build_nc()
    return _CACHED_NC


def kernel(x, w_qkv, w_proj, b_proj, _trace=False, _tmpdir=None):
    x = np.asarray(x, dtype=np.float32)
    w_qkv = np.asarray(w_qkv, dtype=np.float32)
    w_proj = np.asarray(w_proj, dtype=np.float32)
    b_proj = np.asarray(b_proj, dtype=np.float32)

    # Host-side layout prep: transpose + fp16 casts + per-core rotation.
    xT = [np.ascontiguousarray(x[b].T).astype(np.float16) for b in range(B)]
    wq = np.ascontiguousarray(w_qkv[:, 0:D]).astype(np.float16)
    wk = np.ascontiguousarray(w_qkv[:, D:2 * D]).astype(np.float16)
    wv = np.ascontiguousarray(w_qkv[:, 2 * D:3 * D]).astype(np.float16)
    wp = w_proj.astype(np.float16)
    bias = np.ascontiguousarray(
        np.broadcast_to(b_proj, (128, D))
    ).astype(np.float32)

    in_maps = []
    for c in range(NCORES):
        b = c // (NCORES // B)
        qofs = (c % (NCORES // B)) * QS
        xt_rot = np.ascontiguousarray(np.roll(xT[b], -qofs, axis=1))
        in_maps.append(
            {
                "xt": xt_rot,
                "wq": wq,
                "wk": wk,
                "wv": wv,
                "wp": wp,
                "bias": bias,
            }
        )

    nc = get_nc()
    res = bass_utils.run_bass_kernel_spmd(
        nc,
        in_maps,
        core_ids=list(range(NCORES)),
        trace=_trace,
        tmpdir=_tmpdir,
    )

    out = np.empty((B, T, D), dtype=np.float32)
    for c in range(NCORES):
        b = c // (NCORES // B)
        qofs = (c % (NCORES // B)) * QS
        out[b, qofs:qofs + QS] = res.results[c]["out"]
    if _trace:
        kernel._last_results = res
    return out

